# revision 65
# baseline (speedup 1.0000x reference)
"""Trainium2 Bass kernel for the CherryAllocation NAGNN (grid GIN + MLP head).

Self-contained: hardcodes shapes/sharding. Data-parallel over batch:
64 samples -> 8 NeuronCores x 8 samples. Weights replicated.

Math per sample (grid 32x32, N=1024 nodes):
  mask = obs[:1024] != 0 ; x = obs[1024:].reshape(1024, 32)
  h0 = x
  for l in 0..3:  agg = sum of 4-neighbor h ; h = relu(LN(agg @ Wl + bl) * g + be)
  xc = concat([x, h1, h2, h3, h4])  # [1024, 1056]
  z  = relu(BN(xc @ W1 + b1))       # BN eval-mode affine
  y  = z @ W2 + b2 ; out = where(mask, y, -1e7)

Implementation notes:
 - all matmul operands bf16, pre-converted on CPU (no converting DMAs).
 - LN mean is folded into the weights (W' = W - rowmean(W)), so z is
   centered by construction; LN reduces to z * rsqrt(mean(z^2) + eps),
   computed with one vector tensor_tensor_reduce per z block, sqrt on the
   act engine and vector reciprocal.  The normalize multiply runs on the
   act engine (per-partition scale) as the PSUM->SBUF copy.
 - activations feature-major (FM) [feat, tok]; grid aggregation fused into
   the matmul PSUM accumulation: vertical +-32-token shifts via shifted
   stationary-operand slices over zero guard bands; horizontal +-1 neighbors
   pre-summed on GpSimd (hh).  hh-dependent matmuls are emitted last so the
   GpSimd latency hides under the up/down matmuls.
 - layer 0 packs all three shift variants into the partition dim (x3,
   k=96) so it is ONE matmul per token block against a 3x-stacked W0.
 - transposes are plain matmuls against the identity (normal stationary
   load) rather than transpose-mode LDWEIGHTS, which does not overlap with
   the preceding matmul.
 - the per-block-pair normalize multiplies are split across the act and
   vector engines so they run in parallel.
 - samples processed in interleaved pairs so one sample's matmuls cover the
   other's LayerNorm chain; the next pair's x load/transpose is emitted
   inside the current pair's W1 phase; masks prefetched at kernel start.
"""

import numpy as np

import concourse.bass as bass
import concourse.bacc as bacc
import concourse.mybir as mybir
import concourse.tile as tile
from concourse.bass_utils import run_bass_kernel_spmd
from concourse.masks import make_identity

FP = mybir.dt.float32
BF = mybir.dt.bfloat16
AF = mybir.ActivationFunctionType
OP = mybir.AluOpType

GRID = 32
NN = 1024            # nodes per sample
F_IN = 32
H = 256
B = 64
S = 8                # samples per core
NCORE = 8
NB = 8               # 128-token blocks per sample
OBS_W = NN + NN * F_IN   # 33792
MIN_VAL = -10000000.0
EPS_LN = 1e-5
EPS_BN = 1e-5
PAD = 32             # token guard band for vertical shifts
HW = NN + 2 * PAD    # 1088, padded token width per feature-half

PROFILE = False
LAST_EXEC_NS = None
TRACE_KWARGS = {}


def _build(has_gin_bias: bool, b2_val: float) -> bass.Bass:
    nc = bacc.Bacc("TRN2", target_bir_lowering=False, debug=False)

    xf = nc.declare_dram_parameter("xf", [S, 128, 256], BF, isOutput=False)
    msk = nc.declare_dram_parameter("msk", [S, NN], FP, isOutput=False)
    w0 = nc.declare_dram_parameter("w0", [3 * F_IN, H], BF, isOutput=False)
    ws = nc.declare_dram_parameter("ws", [3, 2, 128, H], BF, isOutput=False)
    w1x = nc.declare_dram_parameter("w1x", [F_IN, 512], BF, isOutput=False)
    w1h = nc.declare_dram_parameter("w1h", [8, 128, 512], BF, isOutput=False)
    w2 = nc.declare_dram_parameter("w2", [4, 128], BF, isOutput=False)
    gg = nc.declare_dram_parameter("gg", [4, H], FP, isOutput=False)
    bb = nc.declare_dram_parameter("bb", [4, H], FP, isOutput=False)
    bns = nc.declare_dram_parameter("bns", [512], FP, isOutput=False)
    bnt = nc.declare_dram_parameter("bnt", [512], FP, isOutput=False)
    if has_gin_bias:
        gbias = nc.declare_dram_parameter("gbias", [4, H], BF, isOutput=False)
    y_out = nc.declare_dram_parameter("y", [S, NN], FP, isOutput=True)

    from contextlib import ExitStack

    with tile.TileContext(nc) as tc, ExitStack() as ctx:
        wp = ctx.enter_context(tc.tile_pool(name="w", bufs=1))
        px = ctx.enter_context(tc.tile_pool(name="px", bufs=4))
        ph = ctx.enter_context(tc.tile_pool(name="ph", bufs=2))
        pst = ctx.enter_context(tc.tile_pool(name="pst", bufs=8))
        pfin = ctx.enter_context(tc.tile_pool(name="pfin", bufs=1))
        pz = ctx.enter_context(tc.tile_pool(name="pz", bufs=4, space="PSUM"))
        ptf = ctx.enter_context(tc.tile_pool(name="ptf", bufs=4, space="PSUM"))

        # first-pair x loads issued before anything else: the x_nm DMA is
        # the startup critical path for the first transposes.
        first_xnm = {}
        for s in (0, 1):
            x_nm = px.tile([128, 256], BF, tag="xnm", name=f"xnm0_{s}")
            nc.sync.dma_start(x_nm[0:64, :], xf[s, 0:64])
            nc.gpsimd.dma_start(x_nm[64:128, :], xf[s, 64:128])
            first_xnm[s] = x_nm

        # ---- constants / weights in SBUF ----
        ident = wp.tile([128, 128], BF, tag="id")
        make_identity(nc, ident[:])
        eps_sb = wp.tile([128, 1], FP, tag="eps")
        nc.gpsimd.memset(eps_sb[:], EPS_LN)

        # weight DMAs trigger from the scalar queue (idle at startup) so
        # they don't serialize with gpsimd memsets / hh builds.
        w0_sb = wp.tile([3 * F_IN, H], BF, tag="w0")
        nc.scalar.dma_start(w0_sb[:], w0[:, :])

        wl_sb = []
        for l in range(3):
            t = wp.tile([128, 2 * H], BF, tag=f"wl{l}")
            nc.scalar.dma_start(
                t[:].rearrange("p (k n) -> p k n", k=2),
                ws[l].rearrange("k p n -> p k n"),
            )
            wl_sb.append(t)

        w1x_sb = wp.tile([F_IN, 512], BF, tag="w1x")
        nc.scalar.dma_start(w1x_sb[:], w1x[:, :])
        w1h_sb = wp.tile([128, 8 * 512], BF, tag="w1h")
        nc.scalar.dma_start(
            w1h_sb[:].rearrange("p (j m) -> p j m", j=8),
            w1h[:, :, :].rearrange("j p m -> p j m"),
        )
        w2_sb = wp.tile([128, 4], BF, tag="w2")
        nc.scalar.dma_start(w2_sb[:], w2[:, :].rearrange("k p -> p k"))

        gg_sb = wp.tile([128, 8], FP, tag="gg")
        nc.sync.dma_start(
            gg_sb[:].rearrange("p (l c) -> p l c", c=2),
            gg[:, :].rearrange("l (c p) -> p l c", p=128),
        )
        bb_sb = wp.tile([128, 8], FP, tag="bb")
        nc.sync.dma_start(
            bb_sb[:].rearrange("p (l c) -> p l c", c=2),
            bb[:, :].rearrange("l (c p) -> p l c", p=128),
        )
        bns_sb = wp.tile([128, 4], FP, tag="bns")
        nc.sync.dma_start(bns_sb[:], bns[:].rearrange("(m p) -> p m", p=128))
        bnt_sb = wp.tile([128, 4], FP, tag="bnt")
        nc.sync.dma_start(bnt_sb[:], bnt[:].rearrange("(m p) -> p m", p=128))

        if has_gin_bias:
            ones1 = wp.tile([1, 128], BF, tag="ones1")
            nc.gpsimd.memset(ones1[:].bitcast(mybir.dt.uint16), 0x3F80)
            gb_sb = wp.tile([1, 4 * H], BF, tag="gb")
            nc.gpsimd.dma_start(
                gb_sb[:].rearrange("q (l n) -> q l n", l=4), gbias[:, :]
            )

        def build_hh(dst, src, eng):
            """dst[f, t] = src[f, left(t)] + src[f, right(t)] over [*, NN]."""
            sv = src.rearrange("p (r c) -> p r c", c=GRID)
            dv = dst.rearrange("p (r c) -> p r c", c=GRID)
            eng.tensor_add(dv[:, :, 1:31], sv[:, :, 0:30], sv[:, :, 2:32])
            eng.tensor_copy(dv[:, :, 0:1], sv[:, :, 1:2])
            eng.tensor_copy(dv[:, :, 31:32], sv[:, :, 30:31])

        def emit_layer_mms(z, b, lhs_tile, hh_tile, n_kc, rhs_of_kc, l):
            """Fused aggregation matmul group for one token block.

            All four blocks' up/down matmuls are emitted before any hh
            matmul (see layer_mm_phase) so the GpSimd hh build of the
            previous transpose phase gets ~2us of cover."""
            # kc-inner order: the first two matmuls only need chunk 0 of the
            # previous layer's h, giving chunk 1's relu a little more slack
            for i, kc in enumerate(range(n_kc)):
                base = kc * HW + b * 128
                nc.tensor.matmul(  # up neighbors (t-32)
                    z[:, :], lhs_tile[:, base : base + 128],
                    rhs_of_kc(kc), start=(i == 0), stop=False)
                nc.tensor.matmul(  # down neighbors (t+32)
                    z[:, :], lhs_tile[:, base + 64 : base + 192],
                    rhs_of_kc(kc), start=False, stop=False)

        def emit_hh_mms(z, b, hh_tile, n_kc, rhs_of_kc, l):
            for kc in range(n_kc):  # horizontal pair, pre-summed on GpSimd
                nc.tensor.matmul(
                    z[:, :],
                    hh_tile[:, kc * NN + b * 128 : kc * NN + b * 128 + 128],
                    rhs_of_kc(kc),
                    start=False,
                    stop=(kc == n_kc - 1 and not has_gin_bias),
                )
            if has_gin_bias:
                nc.tensor.matmul(
                    z[:, :], ones1[0:1, 0:128],
                    gb_sb[0:1, l * H : (l + 1) * H],
                    start=False, stop=True,
                )

        def prep_x(s):
            """Load x for sample s, transpose to FM with guard bands."""
            if s in first_xnm:
                x_nm = first_xnm.pop(s)
            else:
                x_nm = px.tile([128, 256], BF, tag="xnm")
                nc.sync.dma_start(x_nm[0:64, :], xf[s, 0:64])
                nc.gpsimd.dma_start(x_nm[64:128, :], xf[s, 64:128])
            # x3 packs the three shift variants in the partition dim so
            # layer 0 is ONE k=96 matmul per block against 3x-stacked W0:
            #   rows  0-31: x at col offset  0 (down shift; also W1's x)
            #   rows 32-63: x at col offset 64 (up shift)
            #   rows 64-95: hh at col offset 32
            # block b reads x3[:, 32 + b*128 : 32 + b*128 + 128].
            x3 = px.tile([3 * F_IN, HW], BF, tag="x3")
            nc.gpsimd.memset(x3[0:32, NN:HW].bitcast(mybir.dt.uint16), 0)
            nc.gpsimd.memset(x3[32:64, 0:64].bitcast(mybir.dt.uint16), 0)
            for half in range(2):
                x_tfm = ptf.tile([F_IN, 512], FP, tag="tf")
                for i in range(4):
                    b = half * 4 + i
                    nc.tensor.matmul(
                        x_tfm[:, i * 128 : (i + 1) * 128],
                        x_nm[:, b * F_IN : (b + 1) * F_IN],
                        ident[:], start=True, stop=True,
                    )
                nc.scalar.copy(
                    x3[0:32, half * 512 : (half + 1) * 512], x_tfm[:])
                nc.vector.tensor_copy(
                    x3[32:64, 64 + half * 512 : 64 + (half + 1) * 512],
                    x_tfm[:])
            build_hh(x3[64:96, 32 : 32 + NN], x3[0:32, 0:NN], nc.gpsimd)
            return {"s": s, "x3": x3, "h": []}

        def layer_mm_phase(st, l):
            if l == 0:
                x3 = st["x3"]
            else:
                n_kc = 2
                prev, prev_hh = st["h"][l - 1], st["hh"]
                wl = wl_sb[l - 1]
                rhs_of_kc = lambda kc, wl=wl: wl[:, kc * H : (kc + 1) * H]

            # normalize engine flips per SAMPLE parity: the partner sample's
            # normalizes stay off the act queue entirely, so this sample's
            # relu-affine acts (which gate the next layer) start sooner.
            on_act = (st["s"] % 2 == 0)

            def ln_block(zs, bp, t_nm):
                mvp = pst.tile([128, 4], FP, tag="mv")
                for i, z in enumerate(zs):
                    st6 = pst.tile([128, 6], FP, tag="st6")
                    nc.vector.bn_stats(st6[:], z[:, :])
                    nc.vector.bn_aggr(mvp[:, 2 * i : 2 * i + 2], st6[:])
                sdp = pst.tile([128, 2], FP, tag="sd")
                var_view = mvp[:].rearrange("p (b t) -> p t b", t=2)[:, 1, :]
                nc.scalar.activation(sdp[:], var_view, AF.Sqrt,
                                     bias=eps_sb[:, 0:1], scale=1.0)
                invp = pst.tile([128, 2], FP, tag="inv")
                nc.vector.reciprocal(invp[:], sdp[:])
                for i in range(2):
                    dst = t_nm[:, (2 * bp + i) * H : (2 * bp + i + 1) * H]
                    if on_act:
                        nc.scalar.activation(
                            dst, zs[i][:, :],
                            AF.Copy, bias=0.0, scale=invp[:, i : i + 1],
                        )
                    else:
                        nc.vector.tensor_scalar_mul(
                            dst, zs[i][:, :], invp[:, i : i + 1])

            t_nm = ph.tile([128, NB * H], BF, tag="tnm")
            if l == 0:
                for bp in range(4):
                    zs = []
                    for b in (2 * bp, 2 * bp + 1):
                        z = pz.tile([128, H], FP, tag="z")
                        nc.tensor.matmul(
                            z[:, :],
                            x3[:, 32 + b * 128 : 32 + b * 128 + 128],
                            w0_sb[:, :],
                            start=True, stop=not has_gin_bias,
                        )
                        if has_gin_bias:
                            nc.tensor.matmul(
                                z[:, :], ones1[0:1, 0:128],
                                gb_sb[0:1, 0:H], start=False, stop=True,
                            )
                        zs.append(z)
                    ln_block(zs, bp, t_nm)
            else:
                for bp in range(4):
                    zs = []
                    for b in (2 * bp, 2 * bp + 1):
                        z = pz.tile([128, H], FP, tag="z")
                        emit_layer_mms(z, b, prev, prev_hh, n_kc,
                                       rhs_of_kc, l)
                        emit_hh_mms(z, b, prev_hh, n_kc, rhs_of_kc, l)
                        zs.append(z)
                    ln_block(zs, bp, t_nm)
            st["t_nm"] = t_nm

        def layer_tr_phase(st, l):
            t_nm = st.pop("t_nm")
            h_t = ph.tile([128, 2 * HW], BF, tag=f"h{l}")
            nc.gpsimd.memset(h_t[:, 0:PAD].bitcast(mybir.dt.uint16), 0)
            nc.gpsimd.memset(
                h_t[:, PAD + NN : HW + PAD].bitcast(mybir.dt.uint16), 0)
            nc.gpsimd.memset(
                h_t[:, HW + PAD + NN : 2 * HW].bitcast(mybir.dt.uint16), 0)
            if l < 3:
                hh_t = ph.tile([128, 2 * NN], BF, tag="hh", name="hh_t")
            else:
                hh_t = None
            for c in range(2):
                for half in range(2):
                    tf = ptf.tile([128, 512], FP, tag="tf", name="tfc")
                    for i in range(4):
                        b = half * 4 + i
                        nc.tensor.matmul(
                            tf[:, i * 128 : (i + 1) * 128],
                            t_nm[:, b * H + c * 128 : b * H + c * 128 + 128],
                            ident[:], start=True, stop=True,
                        )
                    nc.scalar.activation(
                        h_t[:, c * HW + PAD + half * 512
                            : c * HW + PAD + (half + 1) * 512],
                        tf[:],
                        AF.Relu,
                        scale=gg_sb[:, l * 2 + c : l * 2 + c + 1],
                        bias=bb_sb[:, l * 2 + c : l * 2 + c + 1],
                    )
                if hh_t is not None:
                    build_hh(hh_t[:, c * NN : (c + 1) * NN],
                             h_t[:, c * HW + PAD : c * HW + PAD + NN],
                             nc.gpsimd)
            st["h"].append(h_t)
            if hh_t is not None:
                st["hh"] = hh_t

        def unit_w1(st):
            z_sb = ph.tile([128, 4096], BF, tag="zsb")
            for m in range(4):
                for c2 in range(2):
                    zw1 = ptf.tile([128, 512], FP, tag="tf")
                    for kc in range(9):
                        if kc == 0:
                            lhsT = w1x_sb[:, m * 128 : (m + 1) * 128]
                            rt, roff = st["x3"][0:32, :], 0
                        else:
                            j = kc - 1
                            lhsT = w1h_sb[:, j * 512 + m * 128
                                          : j * 512 + (m + 1) * 128]
                            rt, roff = st["h"][j // 2], (j % 2) * HW + PAD
                        nc.tensor.matmul(
                            zw1[:, :],
                            lhsT,
                            rt[:, roff + c2 * 512 : roff + (c2 + 1) * 512],
                            start=(kc == 0), stop=(kc == 8),
                        )
                    nc.scalar.activation(
                        z_sb[:, m * NN + c2 * 512 : m * NN + (c2 + 1) * 512],
                        zw1[:],
                        AF.Relu,
                        scale=bns_sb[:, m : m + 1],
                        bias=bnt_sb[:, m : m + 1],
                    )
            st["z_sb"] = z_sb

        # all masks prefetched at kernel start (removes DMA latency from
        # each sample's output tail)
        msk_sb = []
        for s in range(S):
            m_s = pfin.tile([1, NN], FP, tag="ms", bufs=8, name=f"msk{s}")
            nc.sync.dma_start(m_s[:], msk[s : s + 1, :])
            msk_sb.append(m_s)

        def unit_w2(st):
            s = st["s"]
            z_sb = st["z_sb"]
            yf = pfin.tile([1, NN], FP, tag="yfin", bufs=2)
            nc.gpsimd.memset(yf[:], MIN_VAL)
            for c2 in range(2):
                yp = pz.tile([1, 512], FP, tag="z")
                for m in range(4):
                    nc.tensor.matmul(
                        yp[0:1, :],
                        w2_sb[:, m : m + 1],
                        z_sb[:, m * NN + c2 * 512 : m * NN + (c2 + 1) * 512],
                        start=(m == 0), stop=(m == 3),
                    )
                if b2_val != 0.0:
                    nc.scalar.add(yp[0:1, :], yp[0:1, :], b2_val)
                # masked select straight from PSUM: no staging copy
                nc.vector.copy_predicated(
                    yf[:, c2 * 512 : (c2 + 1) * 512],
                    msk_sb[s][:, c2 * 512 : (c2 + 1) * 512]
                    .bitcast(mybir.dt.uint32),
                    yp[0:1, :])
            nc.sync.dma_start(y_out[s : s + 1, :], yf[:])

        # ---- interleaved sample pairs: partner matmuls hide LN latency.
        # Both samples' matmul phases are emitted before either sample's
        # transpose phase so the PE instruction stream never waits on the
        # just-issued LayerNorm chain.  The next pair's x prep is emitted
        # between the two W1 units so its DMA + transposes hide under W1
        # matmuls.
        # The next pair's x prep AND its layer-0 matmul+LN phase are emitted
        # inside the current pair's W1 phase, so layer 0's LayerNorm chain
        # (too short to hide itself) runs under ~15us of W1 matmuls.
        sts = [prep_x(0), prep_x(1)]
        for st in sts:
            layer_mm_phase(st, 0)
        for p in range(S // 2):
            for st in sts:
                layer_tr_phase(st, 0)
            for l in range(1, 4):
                for st in sts:
                    layer_mm_phase(st, l)
                for st in sts:
                    layer_tr_phase(st, l)
            unit_w1(sts[0])
            unit_w2(sts[0])
            if p < S // 2 - 1:
                nxt = [prep_x(2 * p + 2), prep_x(2 * p + 3)]
                for st in nxt:
                    layer_mm_phase(st, 0)
            else:
                nxt = None
            unit_w1(sts[1])
            unit_w2(sts[1])
            sts = nxt

    nc.finalize()
    return nc


_BUILD_CACHE = {}


def _get_nc(has_gin_bias: bool, b2_val: float) -> bass.Bass:
    key = (has_gin_bias, float(b2_val))
    if key not in _BUILD_CACHE:
        _BUILD_CACHE[key] = _build(has_gin_bias, b2_val)
    return _BUILD_CACHE[key]


def prep_maps(observations, W0, b0, g0, be0, Ws, bs, gs, bes,
              W1, b1, bn_g, bn_b, bn_m, bn_v, W2, b2, **_ignored):
    import ml_dtypes
    BF_NP = ml_dtypes.bfloat16

    obs = np.asarray(observations, np.float32)
    W0 = np.asarray(W0, np.float64)
    Ws = np.asarray(Ws, np.float64)
    W1 = np.asarray(W1, np.float32)
    W2 = np.asarray(W2, np.float32)
    gg = np.ascontiguousarray(np.stack(
        [np.asarray(g0, np.float32)] + [np.asarray(gs, np.float32)[i] for i in range(3)]))
    bb = np.ascontiguousarray(np.stack(
        [np.asarray(be0, np.float32)] + [np.asarray(bes, np.float32)[i] for i in range(3)]))
    gbias = np.stack(
        [np.asarray(b0, np.float64)] + [np.asarray(bs, np.float64)[i] for i in range(3)])
    has_gin_bias = bool(np.any(gbias != 0.0))
    bn_scale = (np.asarray(bn_g, np.float32)
                / np.sqrt(np.asarray(bn_v, np.float32) + EPS_BN)).astype(np.float32)
    bn_shift = ((np.asarray(b1, np.float32) - np.asarray(bn_m, np.float32)) * bn_scale
                + np.asarray(bn_b, np.float32)).astype(np.float32)
    b2_val = float(np.asarray(b2, np.float32).reshape(-1)[0])

    # Fold the LayerNorm mean subtraction into the GIN weights: with
    # W' = W - rowmean(W) (and centered bias), z = agg @ W' + b' has zero
    # feature-mean, so LN only needs the second moment.
    W0c = W0 - W0.mean(axis=1, keepdims=True)
    Wsc = Ws - Ws.mean(axis=2, keepdims=True)
    gbc = gbias - gbias.mean(axis=1, keepdims=True)

    ws_r = np.ascontiguousarray(Wsc.reshape(3, 2, 128, H).astype(BF_NP))
    # W0 stacked 3x to pair with the x3 packed-shift layout (k=96 matmul)
    w0_r = np.ascontiguousarray(
        np.concatenate([W0c, W0c, W0c], axis=0).astype(BF_NP))
    w1x = np.ascontiguousarray(W1[:F_IN].astype(BF_NP))
    w1h = np.ascontiguousarray(W1[F_IN:].reshape(8, 128, 512).astype(BF_NP))
    w2r = np.ascontiguousarray(W2.reshape(4, 128).astype(BF_NP))

    shared = {
        "w0": w0_r, "ws": ws_r, "w1x": w1x, "w1h": w1h, "w2": w2r,
        "gg": gg, "bb": bb, "bns": bn_scale, "bnt": bn_shift,
    }
    if has_gin_bias:
        shared["gbias"] = np.ascontiguousarray(gbc.astype(BF_NP))
    in_maps = []
    for c in range(NCORE):
        m = dict(shared)
        ob = obs[c * S : (c + 1) * S]
        m["msk"] = np.ascontiguousarray(ob[:, :NN])
        # [S, 1024 tok, 32 f] -> token-block-major [S, 128 p, 8 blk, 32 f]
        m["xf"] = np.ascontiguousarray(
            ob[:, NN:].reshape(S, NB, 128, F_IN).transpose(0, 2, 1, 3)
            .reshape(S, 128, 256).astype(BF_NP))
        in_maps.append(m)
    return in_maps, has_gin_bias, b2_val


def kernel(**inputs) -> np.ndarray:
    global LAST_EXEC_NS
    in_maps, has_gin_bias, b2_val = prep_maps(**inputs)
    nc = _get_nc(has_gin_bias, b2_val)
    res = run_bass_kernel_spmd(
        nc, in_maps, list(range(NCORE)), trace=PROFILE, **TRACE_KWARGS
    )
    LAST_EXEC_NS = res.exec_time_ns
    y = np.concatenate([res.results[c]["y"] for c in range(NCORE)], axis=0)
    return y.reshape(B, NN).astype(np.float32)


# revision 67
# speedup vs baseline: 1.0198x; 1.0198x over previous
"""Trainium2 Bass kernel for the CherryAllocation NAGNN (grid GIN + MLP head).

Self-contained: hardcodes shapes/sharding. Data-parallel over batch:
64 samples -> 8 NeuronCores x 8 samples. Weights replicated.

Math per sample (grid 32x32, N=1024 nodes):
  mask = obs[:1024] != 0 ; x = obs[1024:].reshape(1024, 32)
  h0 = x
  for l in 0..3:  agg = sum of 4-neighbor h ; h = relu(LN(agg @ Wl + bl) * g + be)
  xc = concat([x, h1, h2, h3, h4])  # [1024, 1056]
  z  = relu(BN(xc @ W1 + b1))       # BN eval-mode affine
  y  = z @ W2 + b2 ; out = where(mask, y, -1e7)

Implementation notes:
 - all matmul operands bf16, pre-converted on CPU (no converting DMAs).
 - LN mean is folded into the weights (W' = W - rowmean(W)), so z is
   centered by construction; LN reduces to z * rsqrt(mean(z^2) + eps),
   computed with one vector tensor_tensor_reduce per z block, sqrt on the
   act engine and vector reciprocal.  The normalize multiply runs on the
   act engine (per-partition scale) as the PSUM->SBUF copy.
 - activations feature-major (FM) [feat, tok]; grid aggregation fused into
   the matmul PSUM accumulation: vertical +-32-token shifts via shifted
   stationary-operand slices over zero guard bands; horizontal +-1 neighbors
   pre-summed on GpSimd (hh).  hh-dependent matmuls are emitted last so the
   GpSimd latency hides under the up/down matmuls.
 - layer 0 packs all three shift variants into the partition dim (x3,
   k=96) so it is ONE matmul per token block against a 3x-stacked W0.
 - transposes are plain matmuls against the identity (normal stationary
   load) rather than transpose-mode LDWEIGHTS, which does not overlap with
   the preceding matmul.
 - the per-block-pair normalize multiplies are split across the act and
   vector engines so they run in parallel.
 - samples processed in interleaved pairs so one sample's matmuls cover the
   other's LayerNorm chain; the next pair's x load/transpose is emitted
   inside the current pair's W1 phase; masks prefetched at kernel start.
"""

import numpy as np

import concourse.bass as bass
import concourse.bacc as bacc
import concourse.mybir as mybir
import concourse.tile as tile
from concourse.bass_utils import run_bass_kernel_spmd
from concourse.masks import make_identity

FP = mybir.dt.float32
BF = mybir.dt.bfloat16
AF = mybir.ActivationFunctionType
OP = mybir.AluOpType

GRID = 32
NN = 1024            # nodes per sample
F_IN = 32
H = 256
B = 64
S = 8                # samples per core
NCORE = 8
NB = 8               # 128-token blocks per sample
OBS_W = NN + NN * F_IN   # 33792
MIN_VAL = -10000000.0
EPS_LN = 1e-5
EPS_BN = 1e-5
PAD = 32             # token guard band for vertical shifts
HW = NN + 2 * PAD    # 1088, padded token width per feature-half

PROFILE = False
LAST_EXEC_NS = None
TRACE_KWARGS = {}


def _build(has_gin_bias: bool, b2_val: float) -> bass.Bass:
    nc = bacc.Bacc("TRN2", target_bir_lowering=False, debug=False)

    xf = nc.declare_dram_parameter("xf", [S, 128, 256], BF, isOutput=False)
    msk = nc.declare_dram_parameter("msk", [S, NN], FP, isOutput=False)
    w0 = nc.declare_dram_parameter("w0", [3 * F_IN, H], BF, isOutput=False)
    ws = nc.declare_dram_parameter("ws", [3, 2, 128, H], BF, isOutput=False)
    w1x = nc.declare_dram_parameter("w1x", [F_IN, 512], BF, isOutput=False)
    w1h = nc.declare_dram_parameter("w1h", [8, 128, 512], BF, isOutput=False)
    w2 = nc.declare_dram_parameter("w2", [4, 128], BF, isOutput=False)
    gg = nc.declare_dram_parameter("gg", [4, H], FP, isOutput=False)
    bb = nc.declare_dram_parameter("bb", [4, H], FP, isOutput=False)
    bns = nc.declare_dram_parameter("bns", [512], FP, isOutput=False)
    bnt = nc.declare_dram_parameter("bnt", [512], FP, isOutput=False)
    if has_gin_bias:
        gbias = nc.declare_dram_parameter("gbias", [4, H], BF, isOutput=False)
    y_out = nc.declare_dram_parameter("y", [S, NN], FP, isOutput=True)

    from contextlib import ExitStack

    with tile.TileContext(nc) as tc, ExitStack() as ctx:
        wp = ctx.enter_context(tc.tile_pool(name="w", bufs=1))
        px = ctx.enter_context(tc.tile_pool(name="px", bufs=4))
        ph = ctx.enter_context(tc.tile_pool(name="ph", bufs=2))
        pst = ctx.enter_context(tc.tile_pool(name="pst", bufs=8))
        pfin = ctx.enter_context(tc.tile_pool(name="pfin", bufs=1))
        pz = ctx.enter_context(tc.tile_pool(name="pz", bufs=4, space="PSUM"))
        ptf = ctx.enter_context(tc.tile_pool(name="ptf", bufs=4, space="PSUM"))

        # first-pair x loads issued before anything else: the x_nm DMA is
        # the startup critical path for the first transposes.
        first_xnm = {}
        for s in (0, 1):
            x_nm = px.tile([128, 256], BF, tag="xnm", name=f"xnm0_{s}")
            nc.sync.dma_start(x_nm[0:64, :], xf[s, 0:64])
            nc.gpsimd.dma_start(x_nm[64:128, :], xf[s, 64:128])
            first_xnm[s] = x_nm

        # ---- constants / weights in SBUF ----
        ident = wp.tile([128, 128], BF, tag="id")
        make_identity(nc, ident[:])
        eps_sb = wp.tile([128, 1], FP, tag="eps")
        nc.gpsimd.memset(eps_sb[:], EPS_LN)

        # weight DMAs trigger from the scalar queue (idle at startup) so
        # they don't serialize with gpsimd memsets / hh builds.
        w0_sb = wp.tile([3 * F_IN, H], BF, tag="w0")
        nc.scalar.dma_start(w0_sb[:], w0[:, :])

        wl_sb = []
        for l in range(3):
            t = wp.tile([128, 2 * H], BF, tag=f"wl{l}")
            nc.scalar.dma_start(
                t[:].rearrange("p (k n) -> p k n", k=2),
                ws[l].rearrange("k p n -> p k n"),
            )
            wl_sb.append(t)

        w1x_sb = wp.tile([F_IN, 512], BF, tag="w1x")
        nc.scalar.dma_start(w1x_sb[:], w1x[:, :])
        w1h_sb = wp.tile([128, 8 * 512], BF, tag="w1h")
        nc.scalar.dma_start(
            w1h_sb[:].rearrange("p (j m) -> p j m", j=8),
            w1h[:, :, :].rearrange("j p m -> p j m"),
        )
        w2_sb = wp.tile([128, 4], BF, tag="w2")
        nc.scalar.dma_start(w2_sb[:], w2[:, :].rearrange("k p -> p k"))

        gg_sb = wp.tile([128, 8], FP, tag="gg")
        nc.sync.dma_start(
            gg_sb[:].rearrange("p (l c) -> p l c", c=2),
            gg[:, :].rearrange("l (c p) -> p l c", p=128),
        )
        bb_sb = wp.tile([128, 8], FP, tag="bb")
        nc.sync.dma_start(
            bb_sb[:].rearrange("p (l c) -> p l c", c=2),
            bb[:, :].rearrange("l (c p) -> p l c", p=128),
        )
        bns_sb = wp.tile([128, 4], FP, tag="bns")
        nc.sync.dma_start(bns_sb[:], bns[:].rearrange("(m p) -> p m", p=128))
        bnt_sb = wp.tile([128, 4], FP, tag="bnt")
        nc.sync.dma_start(bnt_sb[:], bnt[:].rearrange("(m p) -> p m", p=128))

        if has_gin_bias:
            ones1 = wp.tile([1, 128], BF, tag="ones1")
            nc.gpsimd.memset(ones1[:].bitcast(mybir.dt.uint16), 0x3F80)
            gb_sb = wp.tile([1, 4 * H], BF, tag="gb")
            nc.gpsimd.dma_start(
                gb_sb[:].rearrange("q (l n) -> q l n", l=4), gbias[:, :]
            )

        def build_hh(dst, src, eng):
            """dst[f, t] = src[f, left(t)] + src[f, right(t)] over [*, NN]."""
            sv = src.rearrange("p (r c) -> p r c", c=GRID)
            dv = dst.rearrange("p (r c) -> p r c", c=GRID)
            eng.tensor_add(dv[:, :, 1:31], sv[:, :, 0:30], sv[:, :, 2:32])
            eng.tensor_copy(dv[:, :, 0:1], sv[:, :, 1:2])
            eng.tensor_copy(dv[:, :, 31:32], sv[:, :, 30:31])

        def emit_layer_mms(z, b, lhs_tile, hh_tile, n_kc, rhs_of_kc, l):
            """Fused aggregation matmul group for one token block.

            All four blocks' up/down matmuls are emitted before any hh
            matmul (see layer_mm_phase) so the GpSimd hh build of the
            previous transpose phase gets ~2us of cover."""
            # kc-inner order: the first two matmuls only need chunk 0 of the
            # previous layer's h, giving chunk 1's relu a little more slack
            for i, kc in enumerate(range(n_kc)):
                base = kc * HW + b * 128
                nc.tensor.matmul(  # up neighbors (t-32)
                    z[:, :], lhs_tile[:, base : base + 128],
                    rhs_of_kc(kc), start=(i == 0), stop=False)
                nc.tensor.matmul(  # down neighbors (t+32)
                    z[:, :], lhs_tile[:, base + 64 : base + 192],
                    rhs_of_kc(kc), start=False, stop=False)

        def emit_hh_mms(z, b, hh_tile, n_kc, rhs_of_kc, l):
            for kc in range(n_kc):  # horizontal pair, pre-summed on GpSimd
                nc.tensor.matmul(
                    z[:, :],
                    hh_tile[:, kc * NN + b * 128 : kc * NN + b * 128 + 128],
                    rhs_of_kc(kc),
                    start=False,
                    stop=(kc == n_kc - 1 and not has_gin_bias),
                )
            if has_gin_bias:
                nc.tensor.matmul(
                    z[:, :], ones1[0:1, 0:128],
                    gb_sb[0:1, l * H : (l + 1) * H],
                    start=False, stop=True,
                )

        def prep_x(s):
            """Load x for sample s, transpose to FM with guard bands."""
            if s in first_xnm:
                x_nm = first_xnm.pop(s)
            else:
                x_nm = px.tile([128, 256], BF, tag="xnm")
                nc.sync.dma_start(x_nm[0:64, :], xf[s, 0:64])
                nc.gpsimd.dma_start(x_nm[64:128, :], xf[s, 64:128])
            # x3 packs the three shift variants in the partition dim so
            # layer 0 is ONE k=96 matmul per block against 3x-stacked W0:
            #   rows  0-31: x at col offset  0 (down shift; also W1's x)
            #   rows 32-63: x at col offset 64 (up shift)
            #   rows 64-95: hh at col offset 32
            # block b reads x3[:, 32 + b*128 : 32 + b*128 + 128].
            x3 = px.tile([3 * F_IN, HW], BF, tag="x3")
            nc.gpsimd.memset(x3[0:32, NN:HW].bitcast(mybir.dt.uint16), 0)
            nc.gpsimd.memset(x3[32:64, 0:64].bitcast(mybir.dt.uint16), 0)
            for half in range(2):
                x_tfm = ptf.tile([F_IN, 512], FP, tag="tf")
                for i in range(4):
                    b = half * 4 + i
                    nc.tensor.matmul(
                        x_tfm[:, i * 128 : (i + 1) * 128],
                        x_nm[:, b * F_IN : (b + 1) * F_IN],
                        ident[:], start=True, stop=True,
                    )
                nc.scalar.copy(
                    x3[0:32, half * 512 : (half + 1) * 512], x_tfm[:])
                nc.vector.tensor_copy(
                    x3[32:64, 64 + half * 512 : 64 + (half + 1) * 512],
                    x_tfm[:])
            build_hh(x3[64:96, 32 : 32 + NN], x3[0:32, 0:NN], nc.gpsimd)
            return {"s": s, "x3": x3, "h": []}

        def layer_mm_phase(st, l):
            if l == 0:
                x3 = st["x3"]
            else:
                n_kc = 2
                prev, prev_hh = st["h"][l - 1], st["hh"]
                wl = wl_sb[l - 1]
                rhs_of_kc = lambda kc, wl=wl: wl[:, kc * H : (kc + 1) * H]

            def ln_block(zs, bp, t_nm):
                mvp = pst.tile([128, 4], FP, tag="mv")
                for i, z in enumerate(zs):
                    st6 = pst.tile([128, 6], FP, tag="st6")
                    nc.vector.bn_stats(st6[:], z[:, :])
                    nc.vector.bn_aggr(mvp[:, 2 * i : 2 * i + 2], st6[:])
                sdp = pst.tile([128, 2], FP, tag="sd")
                var_view = mvp[:].rearrange("p (b t) -> p t b", t=2)[:, 1, :]
                nc.scalar.activation(sdp[:], var_view, AF.Sqrt,
                                     bias=eps_sb[:, 0:1], scale=1.0)
                invp = pst.tile([128, 2], FP, tag="inv")
                nc.vector.reciprocal(invp[:], sdp[:])
                # normalize: one z on the act engine, the other on vector,
                # so the two multiplies run in parallel
                nc.scalar.activation(
                    t_nm[:, 2 * bp * H : (2 * bp + 1) * H], zs[0][:, :],
                    AF.Copy, bias=0.0, scale=invp[:, 0:1],
                )
                nc.vector.tensor_scalar_mul(
                    t_nm[:, (2 * bp + 1) * H : (2 * bp + 2) * H],
                    zs[1][:, :], invp[:, 1:2],
                )

            t_nm = ph.tile([128, NB * H], BF, tag="tnm")
            if l == 0:
                for bp in range(4):
                    zs = []
                    for b in (2 * bp, 2 * bp + 1):
                        z = pz.tile([128, H], FP, tag="z")
                        nc.tensor.matmul(
                            z[:, :],
                            x3[:, 32 + b * 128 : 32 + b * 128 + 128],
                            w0_sb[:, :],
                            start=True, stop=not has_gin_bias,
                        )
                        if has_gin_bias:
                            nc.tensor.matmul(
                                z[:, :], ones1[0:1, 0:128],
                                gb_sb[0:1, 0:H], start=False, stop=True,
                            )
                        zs.append(z)
                    ln_block(zs, bp, t_nm)
            else:
                for bp in range(4):
                    zs = []
                    for b in (2 * bp, 2 * bp + 1):
                        z = pz.tile([128, H], FP, tag="z")
                        emit_layer_mms(z, b, prev, prev_hh, n_kc,
                                       rhs_of_kc, l)
                        emit_hh_mms(z, b, prev_hh, n_kc, rhs_of_kc, l)
                        zs.append(z)
                    ln_block(zs, bp, t_nm)
            st["t_nm"] = t_nm

        def layer_tr_phase(st, l):
            t_nm = st.pop("t_nm")
            h_t = ph.tile([128, 2 * HW], BF, tag=f"h{l}")
            nc.gpsimd.memset(h_t[:, 0:PAD].bitcast(mybir.dt.uint16), 0)
            nc.gpsimd.memset(
                h_t[:, PAD + NN : HW + PAD].bitcast(mybir.dt.uint16), 0)
            nc.gpsimd.memset(
                h_t[:, HW + PAD + NN : 2 * HW].bitcast(mybir.dt.uint16), 0)
            if l < 3:
                hh_t = ph.tile([128, 2 * NN], BF, tag="hh", name="hh_t")
            else:
                hh_t = None
            for c in range(2):
                for half in range(2):
                    tf = ptf.tile([128, 512], FP, tag="tf", name="tfc")
                    for i in range(4):
                        b = half * 4 + i
                        nc.tensor.matmul(
                            tf[:, i * 128 : (i + 1) * 128],
                            t_nm[:, b * H + c * 128 : b * H + c * 128 + 128],
                            ident[:], start=True, stop=True,
                        )
                    nc.scalar.activation(
                        h_t[:, c * HW + PAD + half * 512
                            : c * HW + PAD + (half + 1) * 512],
                        tf[:],
                        AF.Relu,
                        scale=gg_sb[:, l * 2 + c : l * 2 + c + 1],
                        bias=bb_sb[:, l * 2 + c : l * 2 + c + 1],
                    )
                if hh_t is not None:
                    build_hh(hh_t[:, c * NN : (c + 1) * NN],
                             h_t[:, c * HW + PAD : c * HW + PAD + NN],
                             nc.gpsimd)
            st["h"].append(h_t)
            if hh_t is not None:
                st["hh"] = hh_t

        def unit_w1(st):
            z_sb = ph.tile([128, 4096], BF, tag="zsb")
            for m in range(4):
                for c2 in range(2):
                    zw1 = ptf.tile([128, 512], FP, tag="tf")
                    for kc in range(9):
                        if kc == 0:
                            lhsT = w1x_sb[:, m * 128 : (m + 1) * 128]
                            rt, roff = st["x3"][0:32, :], 0
                        else:
                            j = kc - 1
                            lhsT = w1h_sb[:, j * 512 + m * 128
                                          : j * 512 + (m + 1) * 128]
                            rt, roff = st["h"][j // 2], (j % 2) * HW + PAD
                        nc.tensor.matmul(
                            zw1[:, :],
                            lhsT,
                            rt[:, roff + c2 * 512 : roff + (c2 + 1) * 512],
                            start=(kc == 0), stop=(kc == 8),
                        )
                    nc.scalar.activation(
                        z_sb[:, m * NN + c2 * 512 : m * NN + (c2 + 1) * 512],
                        zw1[:],
                        AF.Relu,
                        scale=bns_sb[:, m : m + 1],
                        bias=bnt_sb[:, m : m + 1],
                    )
            st["z_sb"] = z_sb

        # all masks prefetched at kernel start (removes DMA latency from
        # each sample's output tail)
        msk_sb = []
        for s in range(S):
            m_s = pfin.tile([1, NN], FP, tag="ms", bufs=8, name=f"msk{s}")
            nc.sync.dma_start(m_s[:], msk[s : s + 1, :])
            msk_sb.append(m_s)

        def unit_w2(st):
            s = st["s"]
            z_sb = st["z_sb"]
            yf = pfin.tile([1, NN], FP, tag="yfin", bufs=2)
            nc.gpsimd.memset(yf[:], MIN_VAL)
            for c2 in range(2):
                yp = pz.tile([1, 512], FP, tag="z")
                for m in range(4):
                    nc.tensor.matmul(
                        yp[0:1, :],
                        w2_sb[:, m : m + 1],
                        z_sb[:, m * NN + c2 * 512 : m * NN + (c2 + 1) * 512],
                        start=(m == 0), stop=(m == 3),
                    )
                if b2_val != 0.0:
                    nc.scalar.add(yp[0:1, :], yp[0:1, :], b2_val)
                # masked select straight from PSUM: no staging copy
                nc.vector.copy_predicated(
                    yf[:, c2 * 512 : (c2 + 1) * 512],
                    msk_sb[s][:, c2 * 512 : (c2 + 1) * 512]
                    .bitcast(mybir.dt.uint32),
                    yp[0:1, :])
            nc.sync.dma_start(y_out[s : s + 1, :], yf[:])

        # ---- interleaved sample pairs: partner matmuls hide LN latency.
        # Both samples' matmul phases are emitted before either sample's
        # transpose phase so the PE instruction stream never waits on the
        # just-issued LayerNorm chain.  The next pair's x prep is emitted
        # between the two W1 units so its DMA + transposes hide under W1
        # matmuls.
        # The next pair's x prep AND its layer-0 matmul+LN phase are emitted
        # inside the current pair's W1 phase, so layer 0's LayerNorm chain
        # (too short to hide itself) runs under ~15us of W1 matmuls.
        sts = [prep_x(0), prep_x(1)]
        for st in sts:
            layer_mm_phase(st, 0)
        for p in range(S // 2):
            for st in sts:
                layer_tr_phase(st, 0)
            for l in range(1, 4):
                for st in sts:
                    layer_mm_phase(st, l)
                for st in sts:
                    layer_tr_phase(st, l)
            unit_w1(sts[0])
            unit_w2(sts[0])
            if p < S // 2 - 1:
                nxt = [prep_x(2 * p + 2), prep_x(2 * p + 3)]
                for st in nxt:
                    layer_mm_phase(st, 0)
            else:
                nxt = None
            unit_w1(sts[1])
            unit_w2(sts[1])
            sts = nxt

    nc.finalize()
    return nc


_BUILD_CACHE = {}


def _get_nc(has_gin_bias: bool, b2_val: float) -> bass.Bass:
    key = (has_gin_bias, float(b2_val))
    if key not in _BUILD_CACHE:
        _BUILD_CACHE[key] = _build(has_gin_bias, b2_val)
    return _BUILD_CACHE[key]


def prep_maps(observations, W0, b0, g0, be0, Ws, bs, gs, bes,
              W1, b1, bn_g, bn_b, bn_m, bn_v, W2, b2, **_ignored):
    import ml_dtypes
    BF_NP = ml_dtypes.bfloat16

    obs = np.asarray(observations, np.float32)
    W0 = np.asarray(W0, np.float64)
    Ws = np.asarray(Ws, np.float64)
    W1 = np.asarray(W1, np.float32)
    W2 = np.asarray(W2, np.float32)
    gg = np.ascontiguousarray(np.stack(
        [np.asarray(g0, np.float32)] + [np.asarray(gs, np.float32)[i] for i in range(3)]))
    bb = np.ascontiguousarray(np.stack(
        [np.asarray(be0, np.float32)] + [np.asarray(bes, np.float32)[i] for i in range(3)]))
    gbias = np.stack(
        [np.asarray(b0, np.float64)] + [np.asarray(bs, np.float64)[i] for i in range(3)])
    has_gin_bias = bool(np.any(gbias != 0.0))
    bn_scale = (np.asarray(bn_g, np.float32)
                / np.sqrt(np.asarray(bn_v, np.float32) + EPS_BN)).astype(np.float32)
    bn_shift = ((np.asarray(b1, np.float32) - np.asarray(bn_m, np.float32)) * bn_scale
                + np.asarray(bn_b, np.float32)).astype(np.float32)
    b2_val = float(np.asarray(b2, np.float32).reshape(-1)[0])

    # Fold the LayerNorm mean subtraction into the GIN weights: with
    # W' = W - rowmean(W) (and centered bias), z = agg @ W' + b' has zero
    # feature-mean, so LN only needs the second moment.
    W0c = W0 - W0.mean(axis=1, keepdims=True)
    Wsc = Ws - Ws.mean(axis=2, keepdims=True)
    gbc = gbias - gbias.mean(axis=1, keepdims=True)

    ws_r = np.ascontiguousarray(Wsc.reshape(3, 2, 128, H).astype(BF_NP))
    # W0 stacked 3x to pair with the x3 packed-shift layout (k=96 matmul)
    w0_r = np.ascontiguousarray(
        np.concatenate([W0c, W0c, W0c], axis=0).astype(BF_NP))
    w1x = np.ascontiguousarray(W1[:F_IN].astype(BF_NP))
    w1h = np.ascontiguousarray(W1[F_IN:].reshape(8, 128, 512).astype(BF_NP))
    w2r = np.ascontiguousarray(W2.reshape(4, 128).astype(BF_NP))

    shared = {
        "w0": w0_r, "ws": ws_r, "w1x": w1x, "w1h": w1h, "w2": w2r,
        "gg": gg, "bb": bb, "bns": bn_scale, "bnt": bn_shift,
    }
    if has_gin_bias:
        shared["gbias"] = np.ascontiguousarray(gbc.astype(BF_NP))
    in_maps = []
    for c in range(NCORE):
        m = dict(shared)
        ob = obs[c * S : (c + 1) * S]
        m["msk"] = np.ascontiguousarray(ob[:, :NN])
        # [S, 1024 tok, 32 f] -> token-block-major [S, 128 p, 8 blk, 32 f]
        m["xf"] = np.ascontiguousarray(
            ob[:, NN:].reshape(S, NB, 128, F_IN).transpose(0, 2, 1, 3)
            .reshape(S, 128, 256).astype(BF_NP))
        in_maps.append(m)
    return in_maps, has_gin_bias, b2_val


def kernel(**inputs) -> np.ndarray:
    global LAST_EXEC_NS
    in_maps, has_gin_bias, b2_val = prep_maps(**inputs)
    nc = _get_nc(has_gin_bias, b2_val)
    res = run_bass_kernel_spmd(
        nc, in_maps, list(range(NCORE)), trace=PROFILE, **TRACE_KWARGS
    )
    LAST_EXEC_NS = res.exec_time_ns
    y = np.concatenate([res.results[c]["y"] for c in range(NCORE)], axis=0)
    return y.reshape(B, NN).astype(np.float32)


# revision 70
# speedup vs baseline: 1.0209x; 1.0010x over previous
"""Trainium2 Bass kernel for the CherryAllocation NAGNN (grid GIN + MLP head).

Self-contained: hardcodes shapes/sharding. Data-parallel over batch:
64 samples -> 8 NeuronCores x 8 samples. Weights replicated.

Math per sample (grid 32x32, N=1024 nodes):
  mask = obs[:1024] != 0 ; x = obs[1024:].reshape(1024, 32)
  h0 = x
  for l in 0..3:  agg = sum of 4-neighbor h ; h = relu(LN(agg @ Wl + bl) * g + be)
  xc = concat([x, h1, h2, h3, h4])  # [1024, 1056]
  z  = relu(BN(xc @ W1 + b1))       # BN eval-mode affine
  y  = z @ W2 + b2 ; out = where(mask, y, -1e7)

Implementation notes:
 - all matmul operands bf16, pre-converted on CPU (no converting DMAs).
 - LN mean is folded into the weights (W' = W - rowmean(W)), so z is
   centered by construction; LN reduces to z * rsqrt(mean(z^2) + eps),
   computed with one vector tensor_tensor_reduce per z block, sqrt on the
   act engine and vector reciprocal.  The normalize multiply runs on the
   act engine (per-partition scale) as the PSUM->SBUF copy.
 - activations feature-major (FM) [feat, tok]; grid aggregation fused into
   the matmul PSUM accumulation: vertical +-32-token shifts via shifted
   stationary-operand slices over zero guard bands; horizontal +-1 neighbors
   pre-summed on GpSimd (hh).  hh-dependent matmuls are emitted last so the
   GpSimd latency hides under the up/down matmuls.
 - layer 0 packs all three shift variants into the partition dim (x3,
   k=96) so it is ONE matmul per token block against a 3x-stacked W0.
 - transposes are plain matmuls against the identity (normal stationary
   load) rather than transpose-mode LDWEIGHTS, which does not overlap with
   the preceding matmul.
 - the per-block-pair normalize multiplies are split across the act and
   vector engines so they run in parallel.
 - samples processed in interleaved pairs so one sample's matmuls cover the
   other's LayerNorm chain; the next pair's x load/transpose is emitted
   inside the current pair's W1 phase; masks prefetched at kernel start.
"""

import numpy as np

import concourse.bass as bass
import concourse.bacc as bacc
import concourse.mybir as mybir
import concourse.tile as tile
from concourse.bass_utils import run_bass_kernel_spmd
from concourse.masks import make_identity

FP = mybir.dt.float32
BF = mybir.dt.bfloat16
AF = mybir.ActivationFunctionType
OP = mybir.AluOpType

GRID = 32
NN = 1024            # nodes per sample
F_IN = 32
H = 256
B = 64
S = 8                # samples per core
NCORE = 8
NB = 8               # 128-token blocks per sample
OBS_W = NN + NN * F_IN   # 33792
MIN_VAL = -10000000.0
EPS_LN = 1e-5
EPS_BN = 1e-5
PAD = 32             # token guard band for vertical shifts
HW = NN + 2 * PAD    # 1088, padded token width per feature-half

PROFILE = False
LAST_EXEC_NS = None
TRACE_KWARGS = {}


def _build(has_gin_bias: bool, b2_val: float) -> bass.Bass:
    nc = bacc.Bacc("TRN2", target_bir_lowering=False, debug=False)

    xf = nc.declare_dram_parameter("xf", [S, 128, 256], BF, isOutput=False)
    msk = nc.declare_dram_parameter("msk", [S, NN], FP, isOutput=False)
    w0 = nc.declare_dram_parameter("w0", [3 * F_IN, H], BF, isOutput=False)
    ws = nc.declare_dram_parameter("ws", [3, 2, 128, H], BF, isOutput=False)
    w1x = nc.declare_dram_parameter("w1x", [F_IN, 512], BF, isOutput=False)
    w1h = nc.declare_dram_parameter("w1h", [8, 128, 512], BF, isOutput=False)
    w2 = nc.declare_dram_parameter("w2", [4, 128], BF, isOutput=False)
    gg = nc.declare_dram_parameter("gg", [4, H], FP, isOutput=False)
    bb = nc.declare_dram_parameter("bb", [4, H], FP, isOutput=False)
    bns = nc.declare_dram_parameter("bns", [512], FP, isOutput=False)
    bnt = nc.declare_dram_parameter("bnt", [512], FP, isOutput=False)
    if has_gin_bias:
        gbias = nc.declare_dram_parameter("gbias", [4, H], BF, isOutput=False)
    y_out = nc.declare_dram_parameter("y", [S, NN], FP, isOutput=True)

    from contextlib import ExitStack

    with tile.TileContext(nc) as tc, ExitStack() as ctx:
        wp = ctx.enter_context(tc.tile_pool(name="w", bufs=1))
        px = ctx.enter_context(tc.tile_pool(name="px", bufs=4))
        ph = ctx.enter_context(tc.tile_pool(name="ph", bufs=2))
        pst = ctx.enter_context(tc.tile_pool(name="pst", bufs=8))
        pfin = ctx.enter_context(tc.tile_pool(name="pfin", bufs=1))
        pz = ctx.enter_context(tc.tile_pool(name="pz", bufs=4, space="PSUM"))
        ptf = ctx.enter_context(tc.tile_pool(name="ptf", bufs=4, space="PSUM"))

        # first-pair x loads issued before anything else: the x_nm DMA is
        # the startup critical path for the first transposes.
        first_xnm = {}
        for s in (0, 1):
            x_nm = px.tile([128, 256], BF, tag="xnm", name=f"xnm0_{s}")
            nc.sync.dma_start(x_nm[0:64, :], xf[s, 0:64])
            nc.gpsimd.dma_start(x_nm[64:128, :], xf[s, 64:128])
            first_xnm[s] = x_nm

        # ---- constants / weights in SBUF ----
        ident = wp.tile([128, 128], BF, tag="id")
        make_identity(nc, ident[:])
        eps_sb = wp.tile([128, 1], FP, tag="eps")
        nc.gpsimd.memset(eps_sb[:], EPS_LN)

        # weight DMAs trigger from the scalar queue (idle at startup) so
        # they don't serialize with gpsimd memsets / hh builds.
        w0_sb = wp.tile([3 * F_IN, H], BF, tag="w0")
        nc.scalar.dma_start(w0_sb[:], w0[:, :])

        wl_sb = []
        for l in range(3):
            t = wp.tile([128, 2 * H], BF, tag=f"wl{l}")
            nc.scalar.dma_start(
                t[:].rearrange("p (k n) -> p k n", k=2),
                ws[l].rearrange("k p n -> p k n"),
            )
            wl_sb.append(t)

        w1x_sb = wp.tile([F_IN, 512], BF, tag="w1x")
        nc.scalar.dma_start(w1x_sb[:], w1x[:, :])
        w1h_sb = wp.tile([128, 8 * 512], BF, tag="w1h")
        nc.scalar.dma_start(
            w1h_sb[:].rearrange("p (j m) -> p j m", j=8),
            w1h[:, :, :].rearrange("j p m -> p j m"),
        )
        w2_sb = wp.tile([128, 4], BF, tag="w2")
        nc.scalar.dma_start(w2_sb[:], w2[:, :].rearrange("k p -> p k"))

        gg_sb = wp.tile([128, 8], FP, tag="gg")
        nc.sync.dma_start(
            gg_sb[:].rearrange("p (l c) -> p l c", c=2),
            gg[:, :].rearrange("l (c p) -> p l c", p=128),
        )
        bb_sb = wp.tile([128, 8], FP, tag="bb")
        nc.sync.dma_start(
            bb_sb[:].rearrange("p (l c) -> p l c", c=2),
            bb[:, :].rearrange("l (c p) -> p l c", p=128),
        )
        bns_sb = wp.tile([128, 4], FP, tag="bns")
        nc.sync.dma_start(bns_sb[:], bns[:].rearrange("(m p) -> p m", p=128))
        bnt_sb = wp.tile([128, 4], FP, tag="bnt")
        nc.sync.dma_start(bnt_sb[:], bnt[:].rearrange("(m p) -> p m", p=128))

        if has_gin_bias:
            ones1 = wp.tile([1, 128], BF, tag="ones1")
            nc.gpsimd.memset(ones1[:].bitcast(mybir.dt.uint16), 0x3F80)
            gb_sb = wp.tile([1, 4 * H], BF, tag="gb")
            nc.gpsimd.dma_start(
                gb_sb[:].rearrange("q (l n) -> q l n", l=4), gbias[:, :]
            )

        def build_hh(dst, src, eng):
            """dst[f, t] = src[f, left(t)] + src[f, right(t)] over [*, NN]."""
            sv = src.rearrange("p (r c) -> p r c", c=GRID)
            dv = dst.rearrange("p (r c) -> p r c", c=GRID)
            eng.tensor_add(dv[:, :, 1:31], sv[:, :, 0:30], sv[:, :, 2:32])
            eng.tensor_copy(dv[:, :, 0:1], sv[:, :, 1:2])
            eng.tensor_copy(dv[:, :, 31:32], sv[:, :, 30:31])

        def emit_layer_mms(z, b, lhs_tile, hh_tile, n_kc, rhs_of_kc, l):
            """Fused aggregation matmul group for one token block.

            All four blocks' up/down matmuls are emitted before any hh
            matmul (see layer_mm_phase) so the GpSimd hh build of the
            previous transpose phase gets ~2us of cover."""
            # kc-inner order: the first two matmuls only need chunk 0 of the
            # previous layer's h, giving chunk 1's relu a little more slack
            for i, kc in enumerate(range(n_kc)):
                base = kc * HW + b * 128
                nc.tensor.matmul(  # up neighbors (t-32)
                    z[:, :], lhs_tile[:, base : base + 128],
                    rhs_of_kc(kc), start=(i == 0), stop=False)
                nc.tensor.matmul(  # down neighbors (t+32)
                    z[:, :], lhs_tile[:, base + 64 : base + 192],
                    rhs_of_kc(kc), start=False, stop=False)

        def emit_hh_mms(z, b, hh_tile, n_kc, rhs_of_kc, l):
            for kc in range(n_kc):  # horizontal pair, pre-summed on GpSimd
                nc.tensor.matmul(
                    z[:, :],
                    hh_tile[:, kc * NN + b * 128 : kc * NN + b * 128 + 128],
                    rhs_of_kc(kc),
                    start=False,
                    stop=(kc == n_kc - 1 and not has_gin_bias),
                )
            if has_gin_bias:
                nc.tensor.matmul(
                    z[:, :], ones1[0:1, 0:128],
                    gb_sb[0:1, l * H : (l + 1) * H],
                    start=False, stop=True,
                )

        def prep_x(s):
            """Load x for sample s, transpose to FM with guard bands."""
            if s in first_xnm:
                x_nm = first_xnm.pop(s)
            else:
                x_nm = px.tile([128, 256], BF, tag="xnm")
                nc.sync.dma_start(x_nm[0:64, :], xf[s, 0:64])
                nc.gpsimd.dma_start(x_nm[64:128, :], xf[s, 64:128])
            # x3 packs the three shift variants in the partition dim so
            # layer 0 is ONE k=96 matmul per block against 3x-stacked W0:
            #   rows  0-31: x at col offset  0 (down shift; also W1's x)
            #   rows 32-63: x at col offset 64 (up shift)
            #   rows 64-95: hh at col offset 32
            # block b reads x3[:, 32 + b*128 : 32 + b*128 + 128].
            x3 = px.tile([3 * F_IN, HW], BF, tag="x3")
            nc.gpsimd.memset(x3[0:32, NN:HW].bitcast(mybir.dt.uint16), 0)
            nc.gpsimd.memset(x3[32:64, 0:64].bitcast(mybir.dt.uint16), 0)
            for half in range(2):
                x_tfm = ptf.tile([F_IN, 512], FP, tag="tf")
                for i in range(4):
                    b = half * 4 + i
                    nc.tensor.matmul(
                        x_tfm[:, i * 128 : (i + 1) * 128],
                        x_nm[:, b * F_IN : (b + 1) * F_IN],
                        ident[:], start=True, stop=True,
                    )
                nc.scalar.copy(
                    x3[0:32, half * 512 : (half + 1) * 512], x_tfm[:])
                nc.vector.tensor_copy(
                    x3[32:64, 64 + half * 512 : 64 + (half + 1) * 512],
                    x_tfm[:])
            build_hh(x3[64:96, 32 : 32 + NN], x3[0:32, 0:NN], nc.gpsimd)
            return {"s": s, "x3": x3, "h": []}

        def layer_mm_phase(st, l, bps=(0, 1, 2, 3)):
            if l == 0:
                x3 = st["x3"]
            else:
                n_kc = 2
                prev, prev_hh = st["h"][l - 1], st["hh"]
                wl = wl_sb[l - 1]
                rhs_of_kc = lambda kc, wl=wl: wl[:, kc * H : (kc + 1) * H]

            def ln_block(zs, bp, t_nm):
                mvp = pst.tile([128, 4], FP, tag="mv")
                for i, z in enumerate(zs):
                    st6 = pst.tile([128, 6], FP, tag="st6")
                    nc.vector.bn_stats(st6[:], z[:, :])
                    nc.vector.bn_aggr(mvp[:, 2 * i : 2 * i + 2], st6[:])
                sdp = pst.tile([128, 2], FP, tag="sd")
                var_view = mvp[:].rearrange("p (b t) -> p t b", t=2)[:, 1, :]
                nc.scalar.activation(sdp[:], var_view, AF.Sqrt,
                                     bias=eps_sb[:, 0:1], scale=1.0)
                invp = pst.tile([128, 2], FP, tag="inv")
                nc.vector.reciprocal(invp[:], sdp[:])
                # normalize: one z on the act engine, the other on vector,
                # so the two multiplies run in parallel
                nc.scalar.activation(
                    t_nm[:, 2 * bp * H : (2 * bp + 1) * H], zs[0][:, :],
                    AF.Copy, bias=0.0, scale=invp[:, 0:1],
                )
                nc.vector.tensor_scalar_mul(
                    t_nm[:, (2 * bp + 1) * H : (2 * bp + 2) * H],
                    zs[1][:, :], invp[:, 1:2],
                )

            if "t_nm" in st:
                t_nm = st["t_nm"]
            else:
                t_nm = ph.tile([128, NB * H], BF, tag="tnm")
                st["t_nm"] = t_nm
            if l == 0:
                for bp in bps:
                    zs = []
                    for b in (2 * bp, 2 * bp + 1):
                        z = pz.tile([128, H], FP, tag="z")
                        nc.tensor.matmul(
                            z[:, :],
                            x3[:, 32 + b * 128 : 32 + b * 128 + 128],
                            w0_sb[:, :],
                            start=True, stop=not has_gin_bias,
                        )
                        if has_gin_bias:
                            nc.tensor.matmul(
                                z[:, :], ones1[0:1, 0:128],
                                gb_sb[0:1, 0:H], start=False, stop=True,
                            )
                        zs.append(z)
                    ln_block(zs, bp, t_nm)
            else:
                for bp in bps:
                    zs = []
                    for b in (2 * bp, 2 * bp + 1):
                        z = pz.tile([128, H], FP, tag="z")
                        emit_layer_mms(z, b, prev, prev_hh, n_kc,
                                       rhs_of_kc, l)
                        emit_hh_mms(z, b, prev_hh, n_kc, rhs_of_kc, l)
                        zs.append(z)
                    ln_block(zs, bp, t_nm)

        def layer_tr_phase(st, l):
            t_nm = st.pop("t_nm")
            h_t = ph.tile([128, 2 * HW], BF, tag=f"h{l}")
            nc.gpsimd.memset(h_t[:, 0:PAD].bitcast(mybir.dt.uint16), 0)
            nc.gpsimd.memset(
                h_t[:, PAD + NN : HW + PAD].bitcast(mybir.dt.uint16), 0)
            nc.gpsimd.memset(
                h_t[:, HW + PAD + NN : 2 * HW].bitcast(mybir.dt.uint16), 0)
            if l < 3:
                hh_t = ph.tile([128, 2 * NN], BF, tag="hh", name="hh_t")
            else:
                hh_t = None
            for c in range(2):
                for half in range(2):
                    tf = ptf.tile([128, 512], FP, tag="tf", name="tfc")
                    for i in range(4):
                        b = half * 4 + i
                        nc.tensor.matmul(
                            tf[:, i * 128 : (i + 1) * 128],
                            t_nm[:, b * H + c * 128 : b * H + c * 128 + 128],
                            ident[:], start=True, stop=True,
                        )
                    nc.scalar.activation(
                        h_t[:, c * HW + PAD + half * 512
                            : c * HW + PAD + (half + 1) * 512],
                        tf[:],
                        AF.Relu,
                        scale=gg_sb[:, l * 2 + c : l * 2 + c + 1],
                        bias=bb_sb[:, l * 2 + c : l * 2 + c + 1],
                    )
                if hh_t is not None:
                    build_hh(hh_t[:, c * NN : (c + 1) * NN],
                             h_t[:, c * HW + PAD : c * HW + PAD + NN],
                             nc.gpsimd)
            st["h"].append(h_t)
            if hh_t is not None:
                st["hh"] = hh_t

        def unit_w1(st):
            z_sb = ph.tile([128, 4096], BF, tag="zsb")
            for m in range(4):
                for c2 in range(2):
                    zw1 = ptf.tile([128, 512], FP, tag="tf")
                    for kc in range(9):
                        if kc == 0:
                            lhsT = w1x_sb[:, m * 128 : (m + 1) * 128]
                            rt, roff = st["x3"][0:32, :], 0
                        else:
                            j = kc - 1
                            lhsT = w1h_sb[:, j * 512 + m * 128
                                          : j * 512 + (m + 1) * 128]
                            rt, roff = st["h"][j // 2], (j % 2) * HW + PAD
                        nc.tensor.matmul(
                            zw1[:, :],
                            lhsT,
                            rt[:, roff + c2 * 512 : roff + (c2 + 1) * 512],
                            start=(kc == 0), stop=(kc == 8),
                        )
                    nc.scalar.activation(
                        z_sb[:, m * NN + c2 * 512 : m * NN + (c2 + 1) * 512],
                        zw1[:],
                        AF.Relu,
                        scale=bns_sb[:, m : m + 1],
                        bias=bnt_sb[:, m : m + 1],
                    )
            st["z_sb"] = z_sb

        # all masks prefetched at kernel start (removes DMA latency from
        # each sample's output tail)
        msk_sb = []
        for s in range(S):
            m_s = pfin.tile([1, NN], FP, tag="ms", bufs=8, name=f"msk{s}")
            nc.sync.dma_start(m_s[:], msk[s : s + 1, :])
            msk_sb.append(m_s)

        def unit_w2(st):
            s = st["s"]
            z_sb = st["z_sb"]
            yf = pfin.tile([1, NN], FP, tag="yfin", bufs=2)
            nc.gpsimd.memset(yf[:], MIN_VAL)
            for c2 in range(2):
                yp = pz.tile([1, 512], FP, tag="z")
                for m in range(4):
                    nc.tensor.matmul(
                        yp[0:1, :],
                        w2_sb[:, m : m + 1],
                        z_sb[:, m * NN + c2 * 512 : m * NN + (c2 + 1) * 512],
                        start=(m == 0), stop=(m == 3),
                    )
                if b2_val != 0.0:
                    nc.scalar.add(yp[0:1, :], yp[0:1, :], b2_val)
                # masked select straight from PSUM: no staging copy
                nc.vector.copy_predicated(
                    yf[:, c2 * 512 : (c2 + 1) * 512],
                    msk_sb[s][:, c2 * 512 : (c2 + 1) * 512]
                    .bitcast(mybir.dt.uint32),
                    yp[0:1, :])
            nc.sync.dma_start(y_out[s : s + 1, :], yf[:])

        # ---- interleaved sample pairs: partner matmuls hide LN latency.
        # Both samples' matmul phases are emitted before either sample's
        # transpose phase so the PE instruction stream never waits on the
        # just-issued LayerNorm chain.  The next pair's x prep is emitted
        # between the two W1 units so its DMA + transposes hide under W1
        # matmuls.
        # The next pair's x prep AND its layer-0 matmul+LN phase are emitted
        # inside the current pair's W1 phase, so layer 0's LayerNorm chain
        # (too short to hide itself) runs under ~15us of W1 matmuls.
        sts = [prep_x(0), prep_x(1)]
        for st in sts:
            layer_mm_phase(st, 0)
        for p in range(S // 2):
            for st in sts:
                layer_tr_phase(st, 0)
            # s1's matmul phase is split around s0's transpose phase: s0's
            # relu-affine acts then queue behind only HALF of s1's norm
            # acts, finishing ~4us earlier and unblocking the next layer,
            # while s1's first-half matmuls still cover s0's LN latency.
            for l in range(1, 4):
                layer_mm_phase(sts[0], l)
                layer_mm_phase(sts[1], l, bps=(0, 1))
                layer_tr_phase(sts[0], l)
                layer_mm_phase(sts[1], l, bps=(2, 3))
                layer_tr_phase(sts[1], l)
            unit_w1(sts[0])
            unit_w2(sts[0])
            if p < S // 2 - 1:
                nxt = [prep_x(2 * p + 2), prep_x(2 * p + 3)]
                for st in nxt:
                    layer_mm_phase(st, 0)
            else:
                nxt = None
            unit_w1(sts[1])
            unit_w2(sts[1])
            sts = nxt

    nc.finalize()
    return nc


_BUILD_CACHE = {}


def _get_nc(has_gin_bias: bool, b2_val: float) -> bass.Bass:
    key = (has_gin_bias, float(b2_val))
    if key not in _BUILD_CACHE:
        _BUILD_CACHE[key] = _build(has_gin_bias, b2_val)
    return _BUILD_CACHE[key]


def prep_maps(observations, W0, b0, g0, be0, Ws, bs, gs, bes,
              W1, b1, bn_g, bn_b, bn_m, bn_v, W2, b2, **_ignored):
    import ml_dtypes
    BF_NP = ml_dtypes.bfloat16

    obs = np.asarray(observations, np.float32)
    W0 = np.asarray(W0, np.float64)
    Ws = np.asarray(Ws, np.float64)
    W1 = np.asarray(W1, np.float32)
    W2 = np.asarray(W2, np.float32)
    gg = np.ascontiguousarray(np.stack(
        [np.asarray(g0, np.float32)] + [np.asarray(gs, np.float32)[i] for i in range(3)]))
    bb = np.ascontiguousarray(np.stack(
        [np.asarray(be0, np.float32)] + [np.asarray(bes, np.float32)[i] for i in range(3)]))
    gbias = np.stack(
        [np.asarray(b0, np.float64)] + [np.asarray(bs, np.float64)[i] for i in range(3)])
    has_gin_bias = bool(np.any(gbias != 0.0))
    bn_scale = (np.asarray(bn_g, np.float32)
                / np.sqrt(np.asarray(bn_v, np.float32) + EPS_BN)).astype(np.float32)
    bn_shift = ((np.asarray(b1, np.float32) - np.asarray(bn_m, np.float32)) * bn_scale
                + np.asarray(bn_b, np.float32)).astype(np.float32)
    b2_val = float(np.asarray(b2, np.float32).reshape(-1)[0])

    # Fold the LayerNorm mean subtraction into the GIN weights: with
    # W' = W - rowmean(W) (and centered bias), z = agg @ W' + b' has zero
    # feature-mean, so LN only needs the second moment.
    W0c = W0 - W0.mean(axis=1, keepdims=True)
    Wsc = Ws - Ws.mean(axis=2, keepdims=True)
    gbc = gbias - gbias.mean(axis=1, keepdims=True)

    ws_r = np.ascontiguousarray(Wsc.reshape(3, 2, 128, H).astype(BF_NP))
    # W0 stacked 3x to pair with the x3 packed-shift layout (k=96 matmul)
    w0_r = np.ascontiguousarray(
        np.concatenate([W0c, W0c, W0c], axis=0).astype(BF_NP))
    w1x = np.ascontiguousarray(W1[:F_IN].astype(BF_NP))
    w1h = np.ascontiguousarray(W1[F_IN:].reshape(8, 128, 512).astype(BF_NP))
    w2r = np.ascontiguousarray(W2.reshape(4, 128).astype(BF_NP))

    shared = {
        "w0": w0_r, "ws": ws_r, "w1x": w1x, "w1h": w1h, "w2": w2r,
        "gg": gg, "bb": bb, "bns": bn_scale, "bnt": bn_shift,
    }
    if has_gin_bias:
        shared["gbias"] = np.ascontiguousarray(gbc.astype(BF_NP))
    in_maps = []
    for c in range(NCORE):
        m = dict(shared)
        ob = obs[c * S : (c + 1) * S]
        m["msk"] = np.ascontiguousarray(ob[:, :NN])
        # [S, 1024 tok, 32 f] -> token-block-major [S, 128 p, 8 blk, 32 f]
        m["xf"] = np.ascontiguousarray(
            ob[:, NN:].reshape(S, NB, 128, F_IN).transpose(0, 2, 1, 3)
            .reshape(S, 128, 256).astype(BF_NP))
        in_maps.append(m)
    return in_maps, has_gin_bias, b2_val


def kernel(**inputs) -> np.ndarray:
    global LAST_EXEC_NS
    in_maps, has_gin_bias, b2_val = prep_maps(**inputs)
    nc = _get_nc(has_gin_bias, b2_val)
    res = run_bass_kernel_spmd(
        nc, in_maps, list(range(NCORE)), trace=PROFILE, **TRACE_KWARGS
    )
    LAST_EXEC_NS = res.exec_time_ns
    y = np.concatenate([res.results[c]["y"] for c in range(NCORE)], axis=0)
    return y.reshape(B, NN).astype(np.float32)
